# revision 16
# baseline (speedup 1.0000x reference)
import sys

sys.path.insert(0, "/opt/trn_rl_repo")

import numpy as np

from concourse import bass, mybir, tile
from concourse import bass_utils
from concourse.masks import make_identity

B, N, K, D = 4, 16384, 32, 64
HALF = 8192
M = HALF * K            # 262144 pairs per core
CHUNK = 8192            # pairs per DMA chunk
NCHUNK = M // CHUNK     # 32
PIECE = 512             # pairs per matmul piece (16 points x 32 nbrs)
NPIECE = CHUNK // PIECE # 16

TRACE = False
LAST_RESULTS = None

_BUILT = None


def _build():
    f32 = mybir.dt.float32
    f16 = mybir.dt.float16
    Copy = mybir.ActivationFunctionType.Copy
    Abs = mybir.ActivationFunctionType.Abs
    add = mybir.AluOpType.add
    mult = mybir.AluOpType.mult

    nc = bass.Bass()
    xgT_d = nc.declare_dram_parameter("xgT", [64, M], f16, False)
    relb_d = nc.declare_dram_parameter("relb", [4, M], f16, False)
    W1b_d = nc.declare_dram_parameter("W1b", [4, 64], f16, False)
    W2a_d = nc.declare_dram_parameter("W2a", [64, 64], f16, False)
    W2c_d = nc.declare_dram_parameter("W2c", [64, 64], f16, False)
    out_d = nc.declare_dram_parameter("out", [128, 64, 64], f32, True)

    with tile.TileContext(nc) as tc:
        frees = []

        def T(shape, dtype, name):
            t, f = tc.tile(shape, dtype, name=name)
            frees.append(f)
            return t

        W1b_sb = T([4, 64], f16, "W1b_sb")
        W2a_sb = T([64, 64], f16, "W2a_sb")
        W2c_sb = T([64, 64], f16, "W2c_sb")
        acc_sb = T([64, HALF], f32, "acc_sb")
        ident = T([128, 128], f32, "ident")

        nc.sync.dma_start(W1b_sb[:, :], W1b_d[:, :])
        nc.sync.dma_start(W2a_sb[:, :], W2a_d[:, :])
        nc.sync.dma_start(W2c_sb[:, :], W2c_d[:, :])
        make_identity(nc, ident[:, :])

        with tc.tile_pool(name="xpool", bufs=2) as xpl, \
             tc.tile_pool(name="rpool", bufs=2) as rpl, \
             tc.tile_pool(name="upool", bufs=2, space="PSUM") as upl, \
             tc.tile_pool(name="wpool", bufs=2, space="PSUM") as wpl, \
             tc.tile_pool(name="spool", bufs=3) as spl, \
             tc.tile_pool(name="apool", bufs=3) as apl, \
             tc.tile_pool(name="tpool", bufs=2) as tpl, \
             tc.tile_pool(name="t2pool", bufs=2) as t2pl:
            for c in range(NCHUNK):
                xg_t = xpl.tile([64, CHUNK], f16, name="xg")
                rl_t = rpl.tile([4, CHUNK], f16, name="rl")
                nc.sync.dma_start(xg_t[:, :], xgT_d[:, c * CHUNK:(c + 1) * CHUNK])
                nc.sync.dma_start(rl_t[:, :], relb_d[:, c * CHUNK:(c + 1) * CHUNK])
                for p2 in range(NPIECE):
                    g = c * NPIECE + p2
                    lo = p2 * PIECE
                    u = upl.tile([64, PIECE], f32, name="u")
                    nc.tensor.matmul(u[:, :], lhsT=W1b_sb[:, :],
                                     rhs=rl_t[:, lo:lo + PIECE],
                                     start=True, stop=True)
                    us = spl.tile([64, PIECE], f16, name="us")
                    ab = apl.tile([64, PIECE], f16, name="ab")
                    nc.scalar.activation(us[:, :], u[:, :], Copy)
                    nc.scalar.activation(ab[:, :], u[:, :], Abs)
                    w = wpl.tile([64, PIECE], f32, name="w")
                    nc.tensor.matmul(w[:, :], lhsT=W2a_sb[:, :], rhs=us[:, :],
                                     start=True, stop=False)
                    nc.tensor.matmul(w[:, :], lhsT=W2c_sb[:, :], rhs=ab[:, :],
                                     start=False, stop=True)
                    t = tpl.tile([64, 16, 32], f16, name="t")
                    t2 = t2pl.tile([64, 16, 16], f16, name="t2")
                    nc.vector.tensor_tensor(t[:, :, :], xg_t[:, lo:lo + PIECE],
                                            w[:, :], mult)
                    nc.vector.tensor_tensor(t2[:, :, :], t[:, :, 0:16],
                                            t[:, :, 16:32], add)
                    nc.vector.tensor_tensor(t[:, :, 0:8], t2[:, :, 0:8],
                                            t2[:, :, 8:16], add)
                    nc.vector.tensor_tensor(t2[:, :, 0:4], t[:, :, 0:4],
                                            t[:, :, 4:8], add)
                    nc.vector.tensor_tensor(t[:, :, 0:2], t2[:, :, 0:2],
                                            t2[:, :, 2:4], add)
                    nc.vector.tensor_tensor(acc_sb[:, g * 16:(g + 1) * 16],
                                            t[:, :, 0:1], t[:, :, 1:2], add)

        out_sb = T([128, 64, 64], f32, "out_sb")
        with tc.tile_pool(name="ppool", bufs=2, space="PSUM") as ppl:
            for tk in range(64):
                pt = ppl.tile([128, 64], f32, name="pt")
                nc.tensor.transpose(pt[:, :], acc_sb[:, tk * 128:(tk + 1) * 128],
                                    ident[0:64, 0:64])
                nc.scalar.activation(out_sb[:, tk:tk + 1, :], pt[:, :], Copy)
        nc.sync.dma_start(out_d[:, :, :], out_sb[:, :, :])
        for f in reversed(frees):
            f()

    import bass_rust
    bass_rust.move_matmul_waits_to_ldweights(nc.m)
    bass_rust.generate_event_semaphores(nc)
    mybir.codegen_inst_isa_subclasses(nc)
    return nc


def _get_nc():
    global _BUILT
    if _BUILT is None:
        _BUILT = _build()
    return _BUILT


def _prep_core(x, pos, nidx, c, W1b, W2a, W2c):
    b, hh = c // 2, c % 2
    sl = slice(hh * HALF, (hh + 1) * HALF)
    idxh = nidx[b, sl]
    xg = x[b][idxh]                                    # [HALF, K, 64]
    rel = pos[b, sl][:, None, :] - pos[b][idxh]        # [HALF, K, 3]
    xgT = np.ascontiguousarray(xg.reshape(M, 64).T.astype(np.float16))
    relb = np.empty((4, M), np.float16)
    relb[0:3] = rel.reshape(M, 3).T
    relb[3] = 1.0
    return dict(xgT=xgT, relb=relb, W1b=W1b, W2a=W2a, W2c=W2c)


def kernel(x, pos, neighbor_idx, W1, b1, W2, b2):
    nc = _get_nc()
    W1b = np.ascontiguousarray(np.vstack([W1, b1[None, :]]).astype(np.float16))
    W2a = np.ascontiguousarray((0.55 * W2).astype(np.float16))
    W2c = np.ascontiguousarray((0.45 * W2).astype(np.float16))
    in_maps = [_prep_core(x, pos, neighbor_idx, c, W1b, W2a, W2c)
               for c in range(8)]
    global LAST_RESULTS
    res = bass_utils.run_bass_kernel_spmd(nc, in_maps, list(range(8)), trace=TRACE)
    LAST_RESULTS = res
    out = np.empty((B, N, D), np.float32)
    for c in range(8):
        b, hh = c // 2, c % 2
        r = np.asarray(res.results[c]["out"])
        out[b, hh * HALF:(hh + 1) * HALF] = r.transpose(1, 0, 2).reshape(HALF, D)
    if np.any(b2):
        for b in range(B):
            s = x[b][neighbor_idx[b]].sum(axis=1)
            out[b] += b2[None, :] * s
    return out


# revision 18
# speedup vs baseline: 1.4748x; 1.4748x over previous
import sys

sys.path.insert(0, "/opt/trn_rl_repo")

import numpy as np

from concourse import bass, mybir, tile
from concourse import bass_utils
from concourse.masks import make_identity

B, N, K, D = 4, 16384, 32, 64
HALF = 8192
M = HALF * K            # 262144 pairs per core
CHUNK = 8192            # pairs per DMA chunk
NCHUNK = M // CHUNK     # 32
GROUP = 1024            # pairs per pipeline group (32 points x 32 nbrs)
NG = CHUNK // GROUP     # 8

TRACE = False
LAST_RESULTS = None

_BUILT = None


def _build():
    f32 = mybir.dt.float32
    f16 = mybir.dt.float16
    Copy = mybir.ActivationFunctionType.Copy
    Abs = mybir.ActivationFunctionType.Abs
    add = mybir.AluOpType.add
    mult = mybir.AluOpType.mult

    nc = bass.Bass()
    xgT_d = nc.declare_dram_parameter("xgT", [64, M], f16, False)
    relb_d = nc.declare_dram_parameter("relb", [4, M], f16, False)
    W1b_d = nc.declare_dram_parameter("W1b", [4, 64], f16, False)
    W2s_d = nc.declare_dram_parameter("W2s", [128, 64], f16, False)
    out_d = nc.declare_dram_parameter("out", [128, 64, 64], f32, True)

    with tile.TileContext(nc) as tc:
        frees = []

        def T(shape, dtype, name):
            t, f = tc.tile(shape, dtype, name=name)
            frees.append(f)
            return t

        W1b_sb = T([4, 64], f16, "W1b_sb")
        W2s_sb = T([128, 64], f16, "W2s_sb")
        acc_sb = T([64, HALF], f32, "acc_sb")
        ident = T([128, 128], f32, "ident")

        nc.sync.dma_start(W1b_sb[:, :], W1b_d[:, :])
        nc.sync.dma_start(W2s_sb[:, :], W2s_d[:, :])
        make_identity(nc, ident[:, :])

        with tc.tile_pool(name="xpool", bufs=2) as xpl, \
             tc.tile_pool(name="rpool", bufs=2) as rpl, \
             tc.tile_pool(name="upool", bufs=2, space="PSUM") as upl, \
             tc.tile_pool(name="wpool", bufs=2, space="PSUM") as wpl, \
             tc.tile_pool(name="spool", bufs=3) as spl, \
             tc.tile_pool(name="tpool", bufs=3) as tpl:
            for c in range(NCHUNK):
                xg_t = xpl.tile([64, CHUNK], f16, name="xg")
                rl_t = rpl.tile([4, CHUNK], f16, name="rl")
                nc.sync.dma_start(xg_t[:, :], xgT_d[:, c * CHUNK:(c + 1) * CHUNK])
                nc.sync.dma_start(rl_t[:, :], relb_d[:, c * CHUNK:(c + 1) * CHUNK])
                for g2 in range(NG):
                    g = c * NG + g2
                    lo = g2 * GROUP
                    u = upl.tile([64, GROUP], f32, name="u")
                    nc.tensor.matmul(u[:, 0:512], lhsT=W1b_sb[:, :],
                                     rhs=rl_t[:, lo:lo + 512],
                                     start=True, stop=True)
                    nc.tensor.matmul(u[:, 512:1024], lhsT=W1b_sb[:, :],
                                     rhs=rl_t[:, lo + 512:lo + 1024],
                                     start=True, stop=True)
                    hs = spl.tile([128, GROUP], f16, name="hs")
                    nc.scalar.activation(hs[0:64, :], u[:, :], Copy)
                    nc.scalar.activation(hs[64:128, :], u[:, :], Abs)
                    w = wpl.tile([64, GROUP], f32, name="w")
                    nc.tensor.matmul(w[:, 0:512], lhsT=W2s_sb[:, :],
                                     rhs=hs[:, 0:512], start=True, stop=True)
                    nc.tensor.matmul(w[:, 512:1024], lhsT=W2s_sb[:, :],
                                     rhs=hs[:, 512:1024], start=True, stop=True)
                    t = tpl.tile([64, 32, 32], f16, name="t")
                    nc.vector.tensor_tensor(t[:, :, :], xg_t[:, lo:lo + GROUP],
                                            w[:, :], mult)
                    nc.vector.tensor_reduce(acc_sb[:, g * 32:(g + 1) * 32],
                                            t[:, :, :],
                                            mybir.AxisListType.X, add)

        out_sb = T([128, 64, 64], f32, "out_sb")
        with tc.tile_pool(name="ppool", bufs=2, space="PSUM") as ppl:
            for tk in range(64):
                pt = ppl.tile([128, 64], f32, name="pt")
                nc.tensor.transpose(pt[:, :], acc_sb[:, tk * 128:(tk + 1) * 128],
                                    ident[0:64, 0:64])
                nc.scalar.activation(out_sb[:, tk:tk + 1, :], pt[:, :], Copy)
        nc.sync.dma_start(out_d[:, :, :], out_sb[:, :, :])
        for f in reversed(frees):
            f()

    import bass_rust
    bass_rust.move_matmul_waits_to_ldweights(nc.m)
    bass_rust.generate_event_semaphores(nc)
    mybir.codegen_inst_isa_subclasses(nc)
    return nc


def _get_nc():
    global _BUILT
    if _BUILT is None:
        _BUILT = _build()
    return _BUILT


def _prep_core(x, pos, nidx, c, W1b, W2s):
    b, hh = c // 2, c % 2
    sl = slice(hh * HALF, (hh + 1) * HALF)
    idxh = nidx[b, sl]
    xg = x[b][idxh]                                    # [HALF, K, 64]
    rel = pos[b, sl][:, None, :] - pos[b][idxh]        # [HALF, K, 3]
    xgT = np.ascontiguousarray(xg.reshape(M, 64).T.astype(np.float16))
    relb = np.empty((4, M), np.float16)
    relb[0:3] = rel.reshape(M, 3).T
    relb[3] = 1.0
    return dict(xgT=xgT, relb=relb, W1b=W1b, W2s=W2s)


def kernel(x, pos, neighbor_idx, W1, b1, W2, b2):
    nc = _get_nc()
    W1b = np.ascontiguousarray(np.vstack([W1, b1[None, :]]).astype(np.float16))
    W2s = np.ascontiguousarray(
        np.vstack([0.55 * W2, 0.45 * W2]).astype(np.float16))
    in_maps = [_prep_core(x, pos, neighbor_idx, c, W1b, W2s)
               for c in range(8)]
    global LAST_RESULTS
    res = bass_utils.run_bass_kernel_spmd(nc, in_maps, list(range(8)), trace=TRACE)
    LAST_RESULTS = res
    out = np.empty((B, N, D), np.float32)
    for c in range(8):
        b, hh = c // 2, c % 2
        r = np.asarray(res.results[c]["out"])
        out[b, hh * HALF:(hh + 1) * HALF] = r.transpose(1, 0, 2).reshape(HALF, D)
    if np.any(b2):
        for b in range(B):
            s = x[b][neighbor_idx[b]].sum(axis=1)
            out[b] += b2[None, :] * s
    return out


# revision 31
# speedup vs baseline: 1.5181x; 1.0293x over previous
import sys

sys.path.insert(0, "/opt/trn_rl_repo")

import numpy as np

from concourse import bass, mybir, tile
from concourse import bass_utils
from concourse.masks import make_identity

B, N, K, D = 4, 16384, 32, 64
HALF = 8192
M = HALF * K            # 262144 pairs per core
CHUNK = 8192            # pairs per DMA chunk
NCHUNK = M // CHUNK     # 32
GROUP = 1024            # pairs per pipeline group (32 points x 32 nbrs)
NG = CHUNK // GROUP     # 8

TRACE = False
LAST_RESULTS = None

_BUILT = None


def _build():
    f32 = mybir.dt.float32
    f16 = mybir.dt.float16
    Copy = mybir.ActivationFunctionType.Copy
    Relu = mybir.ActivationFunctionType.Relu
    add = mybir.AluOpType.add
    mult = mybir.AluOpType.mult

    nc = bass.Bass()
    xgT_d = nc.declare_dram_parameter("xgT", [64, M], f16, False)
    relb_d = nc.declare_dram_parameter("relb", [4, M], f16, False)
    W1b_d = nc.declare_dram_parameter("W1b", [4, 64], f16, False)
    Wstk_d = nc.declare_dram_parameter("Wstk", [68, 64], f16, False)
    out_d = nc.declare_dram_parameter("out", [128, 64, 64], f32, True)

    with tile.TileContext(nc) as tc:
        frees = []

        def T(shape, dtype, name):
            t, f = tc.tile(shape, dtype, name=name)
            frees.append(f)
            return t

        W1b_sb = T([4, 64], f16, "W1b_sb")
        Wstk_sb = T([68, 64], f16, "Wstk_sb")
        acc_sb = T([64, HALF], f32, "acc_sb")
        ident = T([128, 128], f32, "ident")

        nc.sync.dma_start(W1b_sb[:, :], W1b_d[:, :])
        nc.sync.dma_start(Wstk_sb[:, :], Wstk_d[:, :])
        make_identity(nc, ident[:, :])

        with tc.tile_pool(name="xpool", bufs=2) as xpl, \
             tc.tile_pool(name="rpool", bufs=2) as rpl, \
             tc.tile_pool(name="upool", bufs=2, space="PSUM") as upl, \
             tc.tile_pool(name="wpool", bufs=2, space="PSUM") as wpl, \
             tc.tile_pool(name="spool", bufs=3) as spl, \
             tc.tile_pool(name="tpool", bufs=3) as tpl:
            for c in range(NCHUNK):
                xg_t = xpl.tile([64, CHUNK], f16, name="xg")
                rl_t = rpl.tile([4, CHUNK], f16, name="rl")
                nc.sync.dma_start(xg_t[:, :], xgT_d[:, c * CHUNK:(c + 1) * CHUNK])
                nc.sync.dma_start(rl_t[:, :], relb_d[:, c * CHUNK:(c + 1) * CHUNK])
                for g2 in range(NG):
                    g = c * NG + g2
                    lo = g2 * GROUP
                    u = upl.tile([64, GROUP], f32, name="u")
                    nc.tensor.matmul(u[:, 0:512], lhsT=W1b_sb[:, :],
                                     rhs=rl_t[:, lo:lo + 512],
                                     start=True, stop=True)
                    nc.tensor.matmul(u[:, 512:1024], lhsT=W1b_sb[:, :],
                                     rhs=rl_t[:, lo + 512:lo + GROUP],
                                     start=True, stop=True)
                    rs = spl.tile([68, GROUP], f16, name="rs")
                    nc.sync.dma_start(
                        rs[64:68, :],
                        relb_d[:, c * CHUNK + lo:c * CHUNK + lo + GROUP])
                    nc.scalar.activation(rs[0:64, :], u[:, :], Relu)
                    w = wpl.tile([64, GROUP], f32, name="w")
                    nc.tensor.matmul(w[:, 0:512], lhsT=Wstk_sb[:, :],
                                     rhs=rs[:, 0:512], start=True, stop=True)
                    nc.tensor.matmul(w[:, 512:1024], lhsT=Wstk_sb[:, :],
                                     rhs=rs[:, 512:1024], start=True, stop=True)
                    t = tpl.tile([64, 32, 32], f16, name="t")
                    nc.vector.tensor_tensor(t[:, :, :], xg_t[:, lo:lo + GROUP],
                                            w[:, :], mult)
                    nc.vector.tensor_reduce(acc_sb[:, g * 32:(g + 1) * 32],
                                            t[:, :, :],
                                            mybir.AxisListType.X, add)

        out_sb = T([128, 64, 64], f32, "out_sb")
        with tc.tile_pool(name="ppool", bufs=2, space="PSUM") as ppl:
            for tk in range(64):
                pt = ppl.tile([128, 64], f32, name="pt")
                nc.tensor.transpose(pt[:, :], acc_sb[:, tk * 128:(tk + 1) * 128],
                                    ident[0:64, 0:64])
                nc.scalar.activation(out_sb[:, tk:tk + 1, :], pt[:, :], Copy)
        nc.sync.dma_start(out_d[:, :, :], out_sb[:, :, :])
        for f in reversed(frees):
            f()

    import bass_rust
    bass_rust.move_matmul_waits_to_ldweights(nc.m)
    bass_rust.generate_event_semaphores(nc)
    mybir.codegen_inst_isa_subclasses(nc)
    return nc


def _get_nc():
    global _BUILT
    if _BUILT is None:
        _BUILT = _build()
    return _BUILT


def _prep_core(x, pos, nidx, c, W1b, Wstk):
    b, hh = c // 2, c % 2
    sl = slice(hh * HALF, (hh + 1) * HALF)
    idxh = nidx[b, sl]
    xg = x[b][idxh]                                    # [HALF, K, 64]
    rel = pos[b, sl][:, None, :] - pos[b][idxh]        # [HALF, K, 3]
    xgT = np.ascontiguousarray(xg.reshape(M, 64).T.astype(np.float16))
    relb = np.empty((4, M), np.float16)
    relb[0:3] = rel.reshape(M, 3).T
    relb[3] = 1.0
    return dict(xgT=xgT, relb=relb, W1b=W1b, Wstk=Wstk)


def kernel(x, pos, neighbor_idx, W1, b1, W2, b2):
    nc = _get_nc()
    W1b_f = np.vstack([W1, b1[None, :]]).astype(np.float32)
    Wx = W1b_f @ (0.1 * W2.astype(np.float32))         # [4, 64]
    W1b = np.ascontiguousarray(W1b_f.astype(np.float16))
    Wstk = np.ascontiguousarray(
        np.vstack([0.9 * W2, Wx]).astype(np.float16))  # [68, 64]
    in_maps = [_prep_core(x, pos, neighbor_idx, c, W1b, Wstk)
               for c in range(8)]
    global LAST_RESULTS
    res = bass_utils.run_bass_kernel_spmd(nc, in_maps, list(range(8)), trace=TRACE)
    LAST_RESULTS = res
    out = np.empty((B, N, D), np.float32)
    for c in range(8):
        b, hh = c // 2, c % 2
        r = np.asarray(res.results[c]["out"])
        out[b, hh * HALF:(hh + 1) * HALF] = r.transpose(1, 0, 2).reshape(HALF, D)
    if np.any(b2):
        for b in range(B):
            s = x[b][neighbor_idx[b]].sum(axis=1)
            out[b] += b2[None, :] * s
    return out
